# revision 35
# baseline (speedup 1.0000x reference)
"""CharAttention TRN2 kernel: 8-core data-parallel, ragged-packed tokens.

Only the LAST valid character's attention output is consumed, so per word
  q      = x[last] @ Wq                  (one query per word)
  K, V   = x[j] @ Wk, x[j] @ Wv          (valid positions only)
  o      = softmax(q.K / 8) @ V          (no causal mask needed: query is
                                          the last position, attends to all
                                          valid j)
  out    = (o + pos) @ Wp

Raggedness: word lengths are uniform in [1, 32], so ~48% of the padded
[32]-token grid is dead work.  Words are greedily load-balanced across the
8 cores (128 words each) and their valid tokens densely packed into
T = nch*128 token slots per core.  All irregular (per-word-segment) ops
become matmuls against host-built 0/1 matrices:
  qtok[t, :]  = q[word(t), :]        via lhsT = E_c   [w, t]
  o[w, :]    += sum_t pv[t, :]       via lhsT = E_c^T [t, w]
  den[w, h]  += sum_t p[t, h]        via lhsT = E_c^T [t, w]
Orientation: tokens on the PSUM partition dim, so K/V projections use the
packed x chunk [e, t] as the (shared) stationary and the weights as the
moving tensor; every tensor op is regular.

PE cost ~ 352k moving columns/core (vs 606k for the padded baseline).
"""
import os
import sys
import numpy as np

B, W, CC, C = 4, 256, 32, 1024
H, HD = 16, 64
NCORES = 8
WPC = (B * W) // NCORES          # 128 words per core
NE = 8                           # 128-row chunks of the C contraction
NCH_MIN = 17                     # 17*128 = 2176 token slots per core

_cache = {}
LAST_EXEC_NS = None


def _ensure_ntff_hook():
    """Install the axon NTFF profile hook if the image's antenv lacks it.

    bass_utils reads `antenv.axon_hooks.get_axon_ntff_profile_hook()` when
    trace=True under axon; some images ship antenv without that submodule,
    which silently disables HW timing. Recreate it via the boot helper.
    """
    try:
        import antenv.axon_hooks  # noqa: F401
        return
    except ImportError:
        pass
    try:
        import types
        import antenv
        from trn_agent_boot.trn_boot import _ntff_profile_via_ctypes
        hook = _ntff_profile_via_ctypes('/opt/axon/libaxon_pjrt.so')
        if hook is None:
            return
        mod = types.ModuleType("antenv.axon_hooks")
        mod._hook = hook
        mod.set_axon_ntff_profile_hook = lambda h: setattr(mod, "_hook", h)
        mod.get_axon_ntff_profile_hook = lambda: mod._hook
        sys.modules["antenv.axon_hooks"] = mod
        antenv.axon_hooks = mod
    except Exception:
        pass


def _build_nc(nch):
    import concourse.mybir as mybir
    import concourse.tile as tile
    from concourse import bacc

    f32 = mybir.dt.float32
    f32r = mybir.dt.float32r
    fp16 = mybir.dt.float16
    Exp = mybir.ActivationFunctionType.Exp
    AX = mybir.AxisListType.X

    nc = bacc.Bacc("TRN2", target_bir_lowering=False, num_devices=NCORES,
                   debug=False)

    xpack_d = nc.declare_dram_parameter("xpack", [nch, 128, C], fp16,
                                        isOutput=False)
    xlT_d = nc.declare_dram_parameter("xlT", [128, C], fp16, isOutput=False)
    wq_d = nc.declare_dram_parameter("wq", [128, NE * C], fp16,
                                     isOutput=False)
    wk_d = nc.declare_dram_parameter("wk", [128, NE * C], fp16,
                                     isOutput=False)
    wv_d = nc.declare_dram_parameter("wv", [128, NE * C], fp16,
                                     isOutput=False)
    wp_d = nc.declare_dram_parameter("wp", [128, NE * C], fp16,
                                     isOutput=False)
    E_d = nc.declare_dram_parameter("E", [128, nch * 128], fp16,
                                    isOutput=False)
    ET_d = nc.declare_dram_parameter("ET", [128, nch * 128], fp16,
                                     isOutput=False)
    pos_d = nc.declare_dram_parameter("pos", [128, C], f32, isOutput=False)
    ident_d = nc.declare_dram_parameter("ident", [128, 128], f32,
                                        isOutput=False)
    out = nc.declare_dram_parameter("out", [128, C], f32, isOutput=True)

    with tile.TileContext(nc) as tc:
        with tc.tile_pool(name="res", bufs=1) as res, \
             tc.tile_pool(name="xp", bufs=6) as xp, \
             tc.tile_pool(name="work", bufs=2) as work, \
             tc.tile_pool(name="ps", bufs=1, space="PSUM") as ps:

            # ---- resident loads, split per e-tile so packets spread
            # across the DMA engines.  gpsimd queue: xlT, wq, E, ET (the
            # Q/qtok-phase deps), then the xpack chunk stream (pos
            # mid-loop).  sync queue: wk, wv, wp. ----
            xlT_sb = res.tile([128, C], fp16)
            nc.gpsimd.dma_start(xlT_sb[:], xlT_d[:])
            wq_sb = res.tile([128, NE * C], fp16)
            wk_sb = res.tile([128, NE * C], fp16)
            wv_sb = res.tile([128, NE * C], fp16)
            wp_sb = res.tile([128, NE * C], fp16)
            for e in range(NE):
                nc.gpsimd.dma_start(wq_sb[:, e * C:(e + 1) * C],
                                    wq_d[:, e * C:(e + 1) * C])
                nc.sync.dma_start(wk_sb[:, e * C:(e + 1) * C],
                                  wk_d[:, e * C:(e + 1) * C])
                nc.sync.dma_start(wv_sb[:, e * C:(e + 1) * C],
                                  wv_d[:, e * C:(e + 1) * C])
            E_sb = res.tile([128, nch * 128], fp16)
            for h in range(2):
                sl = slice(h * (nch * 64), (h + 1) * (nch * 64))
                nc.gpsimd.dma_start(E_sb[:, sl], E_d[:, sl])
            ET_sb = res.tile([128, nch * 128], fp16)
            for h in range(2):
                sl = slice(h * (nch * 64), (h + 1) * (nch * 64))
                nc.gpsimd.dma_start(ET_sb[:, sl], ET_d[:, sl])
            for e in range(NE):
                nc.sync.dma_start(wp_sb[:, e * C:(e + 1) * C],
                                  wp_d[:, e * C:(e + 1) * C])
            ident_sb = res.tile([128, 128], f32)
            nc.sync.dma_start(ident_sb[:], ident_d[:])

            q_sb = res.tile([128, C], fp16)
            qtok_sb = res.tile([128, nch * C], fp16)

            # ---- PE warm-up: the tensor engine needs ~3us of sustained
            # activity to reach its max p-state, and it would otherwise
            # sit idle during the weight-DMA prefix.  Burn junk matmuls on
            # the first-arrived tile (xlT) so Q and the first chunks run
            # at full clock.  Results are never read. ----
            psW = ps.tile([128, C], f32, tag="psO")
            for r in range(30):
                nc.tensor.matmul(
                    psW[:, 0:512], xlT_sb[:, 0:128], xlT_sb[:, 0:512],
                    start=True, stop=True)

            # ---- Q projection: psq[w, f] = sum_e xlT[e].T @ wq[e] ----
            psq = ps.tile([128, C], f32, tag="psK")
            for e in range(NE):
                for h2 in range(2):
                    nc.tensor.matmul(
                        psq[:, h2 * 512:(h2 + 1) * 512],
                        xlT_sb[:, e * 128:(e + 1) * 128],
                        wq_sb[:, e * C + h2 * 512:e * C + (h2 + 1) * 512],
                        start=(e == 0), stop=(e == NE - 1))
            nc.scalar.copy(q_sb[:], psq[:])

            # ---- qtok[t, f] = E_c.T @ q  (per-token query gather) ----
            for c in range(nch):
                psqt = ps.tile([128, C], f32,
                               tag=("psK" if c % 2 == 0 else "psV"))
                for h2 in range(2):
                    nc.tensor.matmul(
                        psqt[:, h2 * 512:(h2 + 1) * 512],
                        E_sb[:, c * 128:(c + 1) * 128],
                        q_sb[:, h2 * 512:(h2 + 1) * 512],
                        start=True, stop=True)
                nc.scalar.copy(qtok_sb[:, c * C:(c + 1) * C], psqt[:])

            # ---- main chunk loop: K/V projection + scores + softmax +
            # o/den accumulation (o_c emitted one iteration late so the
            # PE never waits on the DVE chain) ----
            pos_sb = res.tile([128, C], f32)

            psO = ps.tile([128, C], f32, tag="psO")
            psDen = ps.tile([128, 16], f32, tag="psDen")
            pvs = [None] * nch
            pts = [None] * nch

            def emit_odan(c):
                pv_c, p_c = pvs[c], pts[c]
                for h2 in range(2):
                    nc.tensor.matmul(
                        psO[:, h2 * 512:(h2 + 1) * 512],
                        ET_sb[:, c * 128:(c + 1) * 128],
                        pv_c[:, h2 * 512:(h2 + 1) * 512],
                        start=(c == 0), stop=(c == nch - 1))
                nc.tensor.matmul(
                    psDen[:], ET_sb[:, c * 128:(c + 1) * 128], p_c[:],
                    start=(c == 0), stop=(c == nch - 1))

            for c in range(nch):
                xp_t = xp.tile([128, C], fp16, tag="xp")
                nc.gpsimd.dma_start(xp_t[:], xpack_d[c])
                if c == 2:
                    nc.gpsimd.dma_start(pos_sb[:], pos_d[:])
                psK = ps.tile([128, C], f32, tag="psK")
                psV = ps.tile([128, C], f32, tag="psV")
                for e in range(NE):
                    lhsT = xp_t[:, e * 128:(e + 1) * 128]
                    for h2 in range(2):
                        nc.tensor.matmul(
                            psK[:, h2 * 512:(h2 + 1) * 512], lhsT,
                            wk_sb[:, e * C + h2 * 512:e * C + (h2 + 1) * 512],
                            start=(e == 0), stop=(e == NE - 1))
                if c > 0:
                    emit_odan(c - 1)
                for e in range(NE):
                    lhsT = xp_t[:, e * 128:(e + 1) * 128]
                    for h2 in range(2):
                        nc.tensor.matmul(
                            psV[:, h2 * 512:(h2 + 1) * 512], lhsT,
                            wv_sb[:, e * C + h2 * 512:e * C + (h2 + 1) * 512],
                            start=(e == 0), stop=(e == NE - 1))
                prod = work.tile([128, C], fp16, tag="prod", bufs=1)
                nc.vector.tensor_mul(prod[:], psK[:],
                                     qtok_sb[:, c * C:(c + 1) * C])
                s_t = work.tile([128, 16], f32, tag="s")
                nc.vector.reduce_sum(
                    s_t[:], prod[:].rearrange("p (h d) -> p h d", d=HD),
                    axis=AX)
                p_t = work.tile([128, 16], fp16, tag="p")
                nc.scalar.activation(p_t[:], s_t[:], Exp,
                                     scale=1.0 / float(np.sqrt(HD)))
                pv = work.tile([128, C], fp16, tag="pv")
                nc.vector.tensor_mul(
                    pv[:].rearrange("p (h d) -> p h d", d=HD),
                    psV[:].rearrange("p (h d) -> p h d", d=HD),
                    p_t[:, :, None].broadcast_to([128, 16, HD]))
                pvs[c], pts[c] = pv, p_t
            emit_odan(nch - 1)

            # ---- normalize + pos, transpose, output projection: pipelined
            # per 128-col block across DVE (normalize), PE (transpose),
            # scalar (copy) and PE (proj accumulation) ----
            rden = work.tile([128, 16], f32, tag="rden")
            nc.vector.reciprocal(rden[:], psDen[:])
            sum_sb = res.tile([128, C], f32)
            sumT_sb = res.tile([128, C], fp16)
            psT = ps.tile([128, C], f32, tag="psV")
            psOut = ps.tile([128, C], f32, tag="psK")
            nc.vector.tensor_mul(
                sum_sb[:].rearrange("p (h d) -> p h d", d=HD),
                psO[:].rearrange("p (h d) -> p h d", d=HD),
                rden[:, :, None].broadcast_to([128, 16, HD]))
            nc.vector.tensor_add(sum_sb[:], sum_sb[:], pos_sb[:])
            for i in range(NE):
                sl = slice(i * 128, (i + 1) * 128)
                nc.tensor.transpose(psT[:, sl], sum_sb[:, sl], ident_sb[:])
            nc.scalar.copy(sumT_sb[:, 0:512], psT[:, 0:512])
            nc.vector.tensor_copy(sumT_sb[:, 512:1024], psT[:, 512:1024])
            for i in range(NE):
                sl = slice(i * 128, (i + 1) * 128)
                for h2 in range(2):
                    nc.tensor.matmul(
                        psOut[:, h2 * 512:(h2 + 1) * 512],
                        sumT_sb[:, sl],
                        wp_sb[:, i * C + h2 * 512:i * C + (h2 + 1) * 512],
                        start=(i == 0), stop=(i == NE - 1))
            out_sb = res.tile([128, C], f32)
            nc.any.tensor_copy(out_sb[:, 0:512], psOut[:, 0:512])
            nc.sync.dma_start(out[:, 0:512], out_sb[:, 0:512])
            nc.any.tensor_copy(out_sb[:, 512:1024], psOut[:, 512:1024])
            nc.sync.dma_start(out[:, 512:1024], out_sb[:, 512:1024])

    nc.finalize()
    return nc


def _prep_inputs(x, attention_mask, pos_emb, attn_w, proj_w):
    x = np.asarray(x, dtype=np.float32)
    attention_mask = np.asarray(attention_mask)
    pos_emb = np.asarray(pos_emb, dtype=np.float32)
    attn_w = np.asarray(attn_w, dtype=np.float32)
    proj_w = np.asarray(proj_w, dtype=np.float32)

    x2 = x.reshape(B * W, CC, C)
    lengths = np.maximum(attention_mask.sum(axis=2).reshape(-1), 1)  # [1024]

    # greedy LPT: longest words first, to the least-loaded core with room
    order = np.argsort(-lengths, kind="stable")
    csum = np.zeros(NCORES, dtype=np.int64)
    ccnt = np.zeros(NCORES, dtype=np.int64)
    assign = [[] for _ in range(NCORES)]
    for g in order:
        k = min((k for k in range(NCORES) if ccnt[k] < WPC),
                key=lambda k: csum[k])
        assign[k].append(int(g))
        csum[k] += int(lengths[g])
        ccnt[k] += 1
    nch = max(NCH_MIN, -(-int(csum.max()) // 128))
    T = nch * 128

    def wlay(wm, dt):  # [C, C] -> [128, NE*C]: row p, col e*C+f = wm[e*128+p, f]
        return np.ascontiguousarray(
            wm.reshape(NE, 128, C).transpose(1, 0, 2).reshape(128, NE * C)
        ).astype(dt)

    wq = wlay(attn_w[:, :C], np.float16)
    wk = wlay(attn_w[:, C:2 * C], np.float16)
    wv = wlay(attn_w[:, 2 * C:], np.float16)
    wp = wlay(proj_w, np.float16)
    ident = np.eye(128, dtype=np.float32)

    in_maps = []
    for core in range(NCORES):
        gw = np.asarray(assign[core])               # [128] global word ids
        L = lengths[gw]                              # [128]
        n_tok = int(L.sum())
        wt = np.repeat(np.arange(WPC), L)            # word slot per token
        jt = np.concatenate([np.arange(l) for l in L])  # char pos per token

        xpf = np.zeros((T, C), dtype=np.float32)
        xpf[:n_tok] = x2[gw[wt], jt]
        xpack = np.ascontiguousarray(
            xpf.reshape(nch, 128, NE, 128).transpose(0, 3, 2, 1)
            .reshape(nch, 128, C)).astype(np.float16)

        xl = x2[gw, L - 1]                           # [128, C]
        xlT = np.ascontiguousarray(                  # [p, e*128+w]
            xl.T.reshape(NE, 128, 128).transpose(1, 0, 2)
            .reshape(128, C)).astype(np.float16)

        E = np.zeros((128, T), dtype=np.float32)     # [w, t]
        E[wt, np.arange(n_tok)] = 1.0
        Ecols = np.ascontiguousarray(E).astype(np.float16)
        ET = np.ascontiguousarray(
            E.T.reshape(nch, 128, 128).transpose(1, 0, 2)
            .reshape(128, nch * 128)).astype(np.float16)

        pos = np.ascontiguousarray(pos_emb[gw % W])  # [128, C]

        in_maps.append({
            "xpack": xpack, "xlT": xlT,
            "wq": wq, "wk": wk, "wv": wv, "wp": wp,
            "E": Ecols, "ET": ET, "pos": pos, "ident": ident,
        })
    return in_maps, assign, nch


def kernel(x, attention_mask, pos_emb, attn_w, proj_w):
    global LAST_EXEC_NS
    from concourse.bass_utils import run_bass_kernel_spmd

    in_maps, assign, nch = _prep_inputs(x, attention_mask, pos_emb,
                                        attn_w, proj_w)
    key = ("nc", nch)
    if key not in _cache:
        _cache[key] = _build_nc(nch)
    nc = _cache[key]
    trace = os.environ.get("KBENCH_TRACE") == "1"
    if trace:
        _ensure_ntff_hook()
    res = run_bass_kernel_spmd(nc, in_maps, core_ids=list(range(NCORES)),
                               trace=trace)
    if trace:
        LAST_EXEC_NS = res.exec_time_ns
    _cache["last_res"] = res
    full = np.empty((B * W, C), dtype=np.float32)
    for core in range(NCORES):
        full[np.asarray(assign[core])] = res.results[core]["out"]
    return np.ascontiguousarray(full.reshape(B, W, C))
